# revision 1
# baseline (speedup 1.0000x reference)
"""Trainium2 Bass kernel for DiffusionProteinFuncModel loss.

Sharding: data-parallel over batch B (4 per core) for q_sample + MHA + MSE;
channel-parallel over D (256 per core) for the per-channel contrastive
losses. Each core emits 4 partial sums; host combines into the scalar loss.

Design:
- All large inputs ship fp8 (e4m3): activations/noise/gT plus weights
  pre-scaled by SW=32 (scale folded into ACT scale params). ~14MB/core
  of host IO vs 112MB for a naive fp32 kernel.
- Wq/Wk/Wo ship as 1/8 column-shards AllGather'd on device (overlapped
  with phases A and D); Wv ships whole so V-compute starts immediately.
- MHA: fp8 DoubleRow matmuls for all four projections (2 k-tiles per
  instruction), V computed directly in natural [tok, d] layout (no
  transposes), exp(logits - 3) keeps E in fp8 range exactly (softmax is
  shift-invariant), softmax denominators via DVE reduce + gpsimd
  partition_all_reduce instead of PE ones-matmuls + broadcast.
- Contrastive: both fl and fs columns pre-normalized on chip so the Gram
  is cosine similarity directly; exp(G/tau) reads PSUM with a constant
  scale; Gram banks interleave with prep chunks and share the startup
  window with the weight AllGather.
"""

import numpy as np
import ml_dtypes

import bass_rust
import concourse.bass as bass
import concourse.bacc as bacc
import concourse.mybir as mybir
from concourse.tile import TileContext
from concourse.bass_utils import run_bass_kernel_spmd

# Problem constants
B, LS, LL, D, H, T = 32, 256, 256, 2048, 16, 1000
TAU = 0.07
SEQ = LS + LL          # 512
DH = D // H            # 128
P = 128
KO = D // P            # 16 partition blocks of the model dim
NCORES = 8
BL = B // NCORES       # 4 batches per core
CHL = D // NCORES      # 256 contrastive channels per core
TB = BL * SEQ // P     # 16 token blocks per core
NG = CHL // 2          # 128 two-channel contrastive groups
CCH = 16               # channels per pre-scale chunk
NCHUNK = CHL // CCH    # 16
ISQ = 1.0 / np.sqrt(DH).astype(np.float32)   # attention scale
SW = 32.0        # fp8 weight pre-scale (host multiplies W by SW)
CEXP = 3.0       # constant logit shift so exp() fits fp8e4 range

F32 = mybir.dt.float32
BF16 = mybir.dt.bfloat16
FP8 = mybir.dt.float8e4
AX = bass_rust.AxisListType.X

USE_ALLGATHER = True


def build_bass():
    nc = bacc.Bacc("TRN2", target_bir_lowering=False, debug=False,
                   enable_asserts=False)

    esT = nc.dram_tensor("esT", [BL, D, LS], FP8, kind="ExternalInput")
    elT = nc.dram_tensor("elT", [BL, D, LL], FP8, kind="ExternalInput")
    nsT = nc.dram_tensor("nsT", [BL, D, SEQ], FP8, kind="ExternalInput")
    sa_d = nc.dram_tensor("sa", [BL, 1], F32, kind="ExternalInput")
    s1m_d = nc.dram_tensor("s1m", [BL, 1], F32, kind="ExternalInput")
    gT_d = nc.dram_tensor("gT", [LS, CHL, 2, B], FP8, kind="ExternalInput")
    eyeX4_d = nc.dram_tensor("eyeX4", [P, 4, P], BF16, kind="ExternalInput")
    mask_d = nc.dram_tensor("maskmat", [P, 4], F32, kind="ExternalInput")
    if USE_ALLGATHER:
        wsh_d = nc.dram_tensor("wsh", [4, D, D // NCORES], FP8,
                               kind="ExternalInput")
    else:
        wfull_d = nc.dram_tensor("wfull", [4, D, D], FP8, kind="ExternalInput")
    pout = nc.dram_tensor("pout", [4, 4], F32, kind="ExternalOutput")

    AF = mybir.ActivationFunctionType
    OP = mybir.AluOpType

    with TileContext(nc) as tc:
        with (
            tc.tile_pool(name="cst", bufs=1) as cst,
            tc.tile_pool(name="acc", bufs=1) as accp,
            tc.tile_pool(name="dram", bufs=1, space="DRAM") as dram,
        ):
            # ---- weight AllGather (issued first; overlaps A + D) ----
            # Split per weight, ordered by first use (V, Q, K, O) so Phase B
            # can start as soon as Wv has arrived.
            CSH = D // NCORES   # columns per weight shard
            if USE_ALLGATHER:
                # Wv ships in full so Phase B's V can start immediately;
                # Wq/Wk/Wo are gathered (slot order q,k,o).
                # two gathers: [q,k] first (gates the heads), o second
                # (only Phase C needs it) — both under the 8.4MB bandwidth
                # floor of the collective, so the sum beats one big gather
                # three gathers in consumer order: [v] gates V-pass0
                # (~128us), [q,k] gates the heads (~353), [o] only Phase C.
                # All under the collective's 8.4MB bandwidth floor.
                w_v = dram.tile([NCORES, D, CSH], FP8)
                wsh_bv = dram.tile([D, CSH], FP8)
                nc.gpsimd.dma_start(wsh_bv[:], wsh_d.ap()[3])
                nc.gpsimd.collective_compute(
                    "AllGather",
                    mybir.AluOpType.bypass,
                    replica_groups=[list(range(NCORES))],
                    ins=[wsh_bv[:]],
                    outs=[w_v[:]],
                )
                w_qk = dram.tile([NCORES, 2, D, CSH], FP8)
                wsh_b1 = dram.tile([2, D, CSH], FP8)
                nc.gpsimd.dma_start(wsh_b1[:], wsh_d.ap()[0:2])
                nc.gpsimd.collective_compute(
                    "AllGather",
                    mybir.AluOpType.bypass,
                    replica_groups=[list(range(NCORES))],
                    ins=[wsh_b1[:]],
                    outs=[w_qk[:]],
                )
                w_o = dram.tile([NCORES, D, CSH], FP8)
                wsh_b2 = dram.tile([D, CSH], FP8)
                nc.gpsimd.dma_start(wsh_b2[:], wsh_d.ap()[2])
                nc.gpsimd.collective_compute(
                    "AllGather",
                    mybir.AluOpType.bypass,
                    replica_groups=[list(range(NCORES))],
                    ins=[wsh_b2[:]],
                    outs=[w_o[:]],
                )

                def w_ap(idx, co, width):
                    # [p, ko, n] for cols [co, co+width) of weight idx
                    s, off = co // CSH, co % CSH
                    assert off + width <= CSH
                    if idx == 2:
                        return w_v[s, :, off:off + width].rearrange(
                            "(ko p) n -> p ko n", p=P)
                    if idx == 3:
                        return w_o[s, :, off:off + width].rearrange(
                            "(ko p) n -> p ko n", p=P)
                    return w_qk[s, idx, :, off:off + width].rearrange(
                        "(ko p) n -> p ko n", p=P)
            else:
                def w_ap(idx, co, width):
                    return wfull_d.ap()[idx, :, co:co + width].rearrange(
                        "(ko p) n -> p ko n", p=P)

            def wdma(dst_tile, idx, co, width):
                # dst tile [P, KO, width]; loads in CSH-wide chunks
                for j, o in enumerate(range(0, width, CSH)):
                    wchunk = min(CSH, width - o)
                    nc.sync.dma_start(dst_tile[:, :, o:o + wchunk],
                                      w_ap(idx, co + o, wchunk))

            ones_col = cst.tile([P, 1], BF16)
            nc.any.memset(ones_col[:], 1.0)
            ones_row = cst.tile([1, P], BF16)
            nc.any.memset(ones_row[:], 1.0)
            ones_col8 = cst.tile([P, 1], FP8)
            nc.any.memset(ones_col8[:], 1.0)
            mcexp_t = cst.tile([P, 1], F32)
            nc.any.memset(mcexp_t[:], -CEXP)
            eyeX4_sb = cst.tile([P, 4, P], BF16)
            nc.sync.dma_start(eyeX4_sb[:], eyeX4_d.ap())
            mask_sb = cst.tile([P, 4], F32)
            nc.sync.dma_start(mask_sb[:], mask_d.ap())
            sa_sb = cst.tile([P, BL], F32)
            nc.sync.dma_start(sa_sb[:], sa_d.ap().rearrange(
                "b one -> (one) b").to_broadcast((P, BL)))
            s1m_sb = cst.tile([P, BL], F32)
            nc.sync.dma_start(s1m_sb[:], s1m_d.ap().rearrange(
                "b one -> (one) b").to_broadcast((P, BL)))

            xsq_acc = accp.tile([P, 2, BL], F32)
            mse_acc = accp.tile([P, KO], F32)
            esum_acc = accp.tile([P, NG], F32)
            d1_all = accp.tile([P, NG], F32)
            for t in (esum_acc,):
                nc.any.memset(t[:], 0.0)

            with tc.tile_pool(name="bigXT", bufs=1) as bigXT:
                ao_f8 = None   # allocated after Phase D (pool opens pre-B)

                if True:
                    xt_bf = bigXT.tile([P, BL, KO, SEQ], FP8)  # x_t^T, din-major

                    # ---------------- Phase A: q_sample ----------------
                    with tc.tile_pool(name="pA", bufs=1) as pA:
                        for b in range(BL):
                            es_t = pA.tile([P, KO, LS], FP8, tag="es")
                            nc.sync.dma_start(es_t[:], esT.ap()[b].rearrange(
                                "(ko p) l -> p ko l", p=P))
                            el_t = pA.tile([P, KO, LL], FP8, tag="el")
                            nc.sync.dma_start(el_t[:], elT.ap()[b].rearrange(
                                "(ko p) l -> p ko l", p=P))
                            ns_t = pA.tile([P, KO, SEQ], FP8, tag="ns")
                            nc.sync.dma_start(ns_t[:], nsT.ap()[b].rearrange(
                                "(ko p) l -> p ko l", p=P))
                            xqa = pA.tile([P, KO, LS], FP8, tag="xqa")
                            nc.scalar.activation(xqa[:], es_t[:], AF.Square,
                                                 accum_out=xsq_acc[:, 0, b:b + 1])
                            xqb = pA.tile([P, KO, LL], FP8, tag="xqb")
                            nc.scalar.activation(xqb[:], el_t[:], AF.Square,
                                                 accum_out=xsq_acc[:, 1, b:b + 1])
                            tmp = pA.tile([P, KO, SEQ], FP8, tag="tmp")
                            nc.vector.tensor_scalar_mul(
                                tmp[:], ns_t[:], s1m_sb[:, b:b + 1])
                            nc.vector.scalar_tensor_tensor(
                                xt_bf[:, b, :, 0:LS], es_t[:], sa_sb[:, b:b + 1],
                                tmp[:, :, 0:LS], OP.mult, OP.add)
                            nc.vector.scalar_tensor_tensor(
                                xt_bf[:, b, :, LS:SEQ], el_t[:], sa_sb[:, b:b + 1],
                                tmp[:, :, LS:SEQ], OP.mult, OP.add)

                    # ------------- Phase D: per-channel contrastive -------------
                    # (no weights needed -> overlaps the AllGather)
                    # gn layout: [p, lo, ch, {fl,fs}, b]; cols pre-scaled 1/||v||
                    _cmV = tc.tile_pool(name="pV", bufs=1)
                    _cmWv = tc.tile_pool(name="pWv", bufs=1)
                    pV = _cmV.__enter__()
                    pWv = _cmWv.__enter__()
                    with (
                        tc.tile_pool(name="pD", bufs=2) as pD,
                        tc.tile_pool(name="pGn", bufs=1) as pGn,
                        tc.tile_pool(name="psG", bufs=2, space="PSUM") as psG,
                    ):
                        # V for head-pass 0 first: only needs the direct-shipped
                        # Wv, so it fills the PE-idle window before Gram banks.
                        HH = H // 2
                        wv_sl = pWv.tile([P, KO, HH * DH], FP8, tag="wv")
                        wdma(wv_sl, 2, 0, HH * DH)
                        v_nat = pV.tile([P, TB, HH * DH], BF16, tag="vn")
                        for tb in range(TB):
                            vb, vq = tb // 4, tb % 4
                            psv = psG.tile([P, 2, SEQ], F32, tag="psv")
                            for k2 in range(KO // 2):
                                for cb in range(2):
                                    nc.tensor.matmul(
                                        psv[:, cb, :],
                                        xt_bf[:, vb, 2 * k2:2 * k2 + 2,
                                              vq * P:(vq + 1) * P],
                                        wv_sl[:, 2 * k2:2 * k2 + 2,
                                              cb * SEQ:(cb + 1) * SEQ],
                                        start=(k2 == 0), stop=(k2 == KO // 2 - 1),
                                        perf_mode=mybir.MatmulPerfMode.DoubleRow)
                            (nc.scalar.copy if tb % 2 else
                             nc.vector.tensor_copy)(v_nat[:, tb, :], psv[:])

                        gn_bf = pGn.tile([P, 2, CHL, 2, B], BF16, tag="gn")
                        gn0 = gn_bf[:, 0].rearrange("p c t b -> p (c t b)")
                        gn1 = gn_bf[:, 1].rearrange("p c t b -> p (c t b)")
                        def emit_bank(bank):
                            gcol = slice(bank * 4, bank * 4 + 4)
                            psg = psG.tile([P, 4, P], F32, tag="psg")
                            for j in range(4):
                                g = bank * 4 + j
                                gs = slice(g * P, (g + 1) * P)
                                nc.tensor.matmul(psg[:, j, :], gn0[:, gs],
                                                 gn0[:, gs],
                                                 start=True, stop=False)
                                nc.tensor.matmul(psg[:, j, :], gn1[:, gs],
                                                 gn1[:, gs],
                                                 start=False, stop=True)
                            dxp = pD.tile([P, 4, P], BF16, tag="dxp")
                            nc.vector.tensor_tensor(dxp[:], psg[:],
                                                    eyeX4_sb[:], OP.mult)
                            nc.vector.reduce_sum(
                                d1_all[:, gcol].rearrange("p g -> p g ()"),
                                dxp[:], axis=AX)
                            ed = pD.tile([P, 4, 32], BF16, tag="ed")
                            nc.scalar.activation(
                                ed[0:64, :, :], psg[0:64, :, 0:32],
                                AF.Exp, scale=float(1.0 / TAU))
                            nc.scalar.activation(
                                ed[64:128, :, :], psg[64:128, :, 64:96],
                                AF.Exp, scale=float(1.0 / TAU))
                            nc.vector.reduce_sum(
                                esum_acc[0:64, gcol].rearrange("p g -> p g ()"),
                                ed[0:64, :, :], axis=AX)
                            nc.vector.reduce_sum(
                                esum_acc[64:128, gcol].rearrange("p g -> p g ()"),
                                ed[64:128, :, :], axis=AX)

                        with (
                            tc.tile_pool(name="pG", bufs=2) as pG,
                            tc.tile_pool(name="pG1", bufs=1) as pG1,
                            tc.tile_pool(name="psN", bufs=1, space="PSUM") as psN,
                            tc.tile_pool(name="psB2", bufs=1, space="PSUM") as psB2,
                        ):
                            for cc in range(NCHUNK):
                                cs = slice(cc * CCH, (cc + 1) * CCH)
                                gf = pG.tile([P, 2, CCH, 2, B], FP8, tag="gf")
                                nc.sync.dma_start(
                                    gf[:], gT_d.ap()[:, cs, :, :].rearrange(
                                        "(lo p) c t b -> p lo c t b", p=P))
                                flsq = pG1.tile([P, 2, CCH, 2, B], BF16,
                                                tag="flsq")
                                nc.vector.tensor_tensor(flsq[:], gf[:], gf[:],
                                                        OP.mult)
                                for half in range(2):
                                    hsl = slice(half * CCH * B,
                                                (half + 1) * CCH * B)
                                    csh = slice(cc * CCH + half * CCH // 2,
                                                cc * CCH + (half + 1) * CCH // 2)
                                    psn = psN.tile([1, CCH * B], F32, tag="psn")
                                    for lo in range(2):
                                        nc.tensor.matmul(
                                            psn[:], ones_col[:],
                                            flsq[:, lo].rearrange(
                                                "p c t b -> p (c t b)")[:, hsl],
                                            start=(lo == 0), stop=(lo == 1))
                                    # 1/||v|| = exp(-0.5*ln(||v||^2))
                                    lnn = pG1.tile([1, CCH * B], F32, tag="lnn")
                                    nc.scalar.activation(lnn[:], psn[:], AF.Ln)
                                    inv2 = pG1.tile([1, CCH * B], BF16,
                                                    tag="inv2")
                                    nc.scalar.activation(inv2[:], lnn[:], AF.Exp,
                                                         scale=-0.5)
                                    psb2 = psB2.tile([P, CCH * B], F32, tag="psb2")
                                    nc.tensor.matmul(psb2[:], ones_row[:], inv2[:],
                                                     start=True, stop=True)
                                    for lo in range(2):
                                        nc.vector.tensor_tensor(
                                            gn_bf[:, lo, csh, :, :],
                                            gf[:, lo, half * CCH // 2:
                                               (half + 1) * CCH // 2],
                                            psb2[:].rearrange(
                                                "p (c t b) -> p c t b",
                                                c=CCH // 2, t=2),
                                            OP.mult)
                                emit_bank(2 * cc)
                                emit_bank(2 * cc + 1)

                    _cmAO = tc.tile_pool(name="bigAO", bufs=1)
                    bigAO = _cmAO.__enter__()
                    ao_f8 = bigAO.tile([P, BL, KO, SEQ], FP8)  # attn out^T
                    with (
                        tc.tile_pool(name="pQ", bufs=2) as pQ,
                        tc.tile_pool(name="pE", bufs=3) as pE,
                        tc.tile_pool(name="pN", bufs=3) as pN,
                        tc.tile_pool(name="ps2", bufs=4, space="PSUM") as ps2,
                    ):
                        import bass_isa
                        for hpass in range(2):
                            if hpass == 1:
                                # V for pass-1 heads
                                wv_sl = pWv.tile([P, KO, HH * DH], FP8, tag="wv")
                                wdma(wv_sl, 2, hpass * HH * DH, HH * DH)
                                v_nat = pV.tile([P, TB, HH * DH], BF16, tag="vn")
                                for tb in range(TB):
                                    vb, vq = tb // 4, tb % 4
                                    psv = ps2.tile([P, 2, SEQ], F32, tag="ps2")
                                    for k2 in range(KO // 2):
                                        for cb in range(2):
                                            nc.tensor.matmul(
                                                psv[:, cb, :],
                                                xt_bf[:, vb, 2 * k2:2 * k2 + 2,
                                                      vq * P:(vq + 1) * P],
                                                wv_sl[:, 2 * k2:2 * k2 + 2,
                                                      cb * SEQ:(cb + 1) * SEQ],
                                                start=(k2 == 0),
                                                stop=(k2 == KO // 2 - 1),
                                                perf_mode=mybir.MatmulPerfMode.DoubleRow)
                                    (nc.scalar.copy if tb % 2 else
                                     nc.vector.tensor_copy)(v_nat[:, tb, :], psv[:])

                            wq_all = pWv.tile([P, KO, HH * DH], FP8, tag="wqa")
                            wdma(wq_all, 0, hpass * HH * DH, HH * DH)
                            wk_all = pWv.tile([P, KO, HH * DH], FP8, tag="wka")
                            wdma(wk_all, 1, hpass * HH * DH, HH * DH)
                            for hh in range(HH):
                                h = hpass * HH + hh
                                wq_t = wq_all[:, :, hh * DH:(hh + 1) * DH]
                                wk_t = wk_all[:, :, hh * DH:(hh + 1) * DH]

                                # Q^T, K^T: [dh, tok], in half-batch psum tiles
                                qT = pQ.tile([P, BL, SEQ], FP8, tag="qT")
                                kT = pQ.tile([P, BL, SEQ], FP8, tag="kT")
                                for wt, dst, cpf in (
                                        (wq_t, qT, nc.vector.tensor_copy),
                                        (wk_t, kT, nc.vector.tensor_copy)):
                                    for j2 in range(2):
                                        psq = ps2.tile([P, 2, SEQ], F32, tag="ps2")
                                        for k2 in range(KO // 2):
                                            for jb in range(2):
                                                nc.tensor.matmul(
                                                    psq[:, jb, :],
                                                    wt[:, 2 * k2:2 * k2 + 2, :],
                                                    xt_bf[:, 2 * j2 + jb,
                                                          2 * k2:2 * k2 + 2, :],
                                                    start=(k2 == 0),
                                                    stop=(k2 == KO // 2 - 1),
                                                    perf_mode=mybir.MatmulPerfMode.DoubleRow)
                                        cpf(dst[:, 2 * j2:2 * j2 + 2, :], psq[:])

                                for bp in range(2):
                                    pso = ps2.tile([P, 2, SEQ], F32, tag="ps2")
                                    for jb in range(2):
                                        b = 2 * bp + jb
                                        # E^T = exp(S^T/sqrt(dh) - CEXP) : [ktok, q]
                                        eT = pE.tile([P, 4, SEQ], BF16, tag="eT")
                                        for half in range(2):
                                            pss = ps2.tile([P, 2, SEQ], F32, tag="ps2")
                                            for j in range(2):
                                                kb = 2 * half + j
                                                nc.tensor.matmul(
                                                    pss[:, j, :],
                                                    kT[:, b, kb * P:(kb + 1) * P],
                                                    qT[:, b, :],
                                                    start=True, stop=True)
                                            nc.scalar.activation(
                                                eT[:, 2 * half:2 * half + 2, :],
                                                pss[:], AF.Exp,
                                                scale=float(ISQ / (SW * SW)),
                                                bias=mcexp_t[:])
                                        # unnormalized out^T accumulate over ktok
                                        for kb in range(4):
                                            nc.tensor.matmul(
                                                pso[:, jb, :],
                                                v_nat[:, 4 * b + kb,
                                                      hh * DH:(hh + 1) * DH],
                                                eT[:, kb, :],
                                                start=(kb == 0), stop=(kb == 3))
                                        # softmax denom: packed bf16 adds
                                        # (DVE 2x mode) + gpsimd allreduce
                                        s2 = pN.tile([P, 2, SEQ], BF16, tag="s2")
                                        nc.vector.tensor_tensor(
                                            s2[:], eT[:, 0:2, :], eT[:, 2:4, :],
                                            OP.add)
                                        s1 = pN.tile([P, SEQ], BF16, tag="s1")
                                        nc.vector.tensor_tensor(
                                            s1[:], s2[:, 0, :], s2[:, 1, :],
                                            OP.add)
                                        rb = pN.tile([P, SEQ], F32, tag="rb")
                                        nc.gpsimd.partition_all_reduce(
                                            rb[:], s1[:], P,
                                            bass_isa.ReduceOp.add)
                                        rcpb = pN.tile([P, SEQ], F32, tag="rcpb")
                                        nc.vector.reciprocal_approx_fast(
                                            rcpb[:], rb[:])
                                        nc.vector.tensor_tensor(
                                            ao_f8[:, b, h, :], pso[:, jb, :],
                                            rcpb[:], OP.mult)

                    # ---------------- Phase C: Wo proj + MSE + xsq ----------------
                    with (
                        tc.tile_pool(name="pWo", bufs=2) as pWo,
                        tc.tile_pool(name="pX", bufs=3) as pX,
                        tc.tile_pool(name="psC", bufs=2, space="PSUM") as psC,
                    ):
                        for do in range(KO):
                            dsl = slice(do * P, (do + 1) * P)
                            wo_f8 = pWo.tile([P, KO, P], FP8, tag="wof8")
                            wdma(wo_f8, 3, do * P, P)
                            psm = psC.tile([P, BL, SEQ], F32, tag="psm")
                            for k2 in range(KO // 2):
                                for b in range(BL):
                                    nc.tensor.matmul(
                                        psm[:, b, :],
                                        wo_f8[:, 2 * k2:2 * k2 + 2, :],
                                        ao_f8[:, b, 2 * k2:2 * k2 + 2, :],
                                        start=(k2 == 0), stop=(k2 == KO // 2 - 1),
                                        perf_mode=mybir.MatmulPerfMode.DoubleRow)
                            xs_t = pX.tile([P, BL, SEQ], FP8, tag="xs")
                            nc.sync.dma_start(
                                xs_t[:, :, 0:LS],
                                esT.ap()[:, dsl, :].rearrange("b p l -> p b l"))
                            nc.sync.dma_start(
                                xs_t[:, :, LS:SEQ],
                                elT.ap()[:, dsl, :].rearrange("b p l -> p b l"))
                            d_t = pX.tile([P, BL, SEQ], BF16, tag="df")
                            nc.vector.scalar_tensor_tensor(
                                d_t[:], psm[:], 1.0 / (SW * SW), xs_t[:],
                                OP.mult, OP.subtract)
                            dmc = pX.tile([P, BL, SEQ], BF16, tag="dmc")
                            nc.scalar.activation(dmc[:], d_t[:], AF.Square,
                                                 accum_out=mse_acc[:, do:do + 1])


                    _cmAO.__exit__(None, None, None)
                    _cmWv.__exit__(None, None, None)
                    _cmV.__exit__(None, None, None)

            # ---------------- Final reduction ----------------
            with (
                tc.tile_pool(name="pF", bufs=1) as pF,
                tc.tile_pool(name="psF", bufs=1, space="PSUM") as psF,
            ):
                lse_t = pF.tile([P, NG], F32)
                nc.scalar.activation(lse_t[:], esum_acc[:], AF.Ln)
                d1s = pF.tile([P, NG], F32)
                nc.vector.tensor_scalar_mul(d1s[:], d1_all[:], float(1.0 / TAU))
                r_t = pF.tile([P, NG], F32)
                nc.vector.tensor_tensor(r_t[:], lse_t[:], d1s[:], OP.subtract)
                nc.vector.tensor_scalar_add(r_t[0:32, :], r_t[0:32, :],
                                            float(-1.0 / TAU))
                nc.vector.tensor_scalar_add(r_t[64:96, :], r_t[64:96, :],
                                            float(-1.0 / TAU))
                colmat = pF.tile([P, 4], F32)
                nc.vector.reduce_sum(
                    colmat[:, 0:1], xsq_acc[:].rearrange("p a b -> p (a b)"),
                    axis=AX)
                nc.vector.reduce_sum(colmat[:, 1:2], mse_acc[:], axis=AX)
                mcol = pF.tile([P, 1], F32)
                nc.vector.reduce_sum(mcol[:], r_t[:], axis=AX)
                nc.vector.tensor_copy(colmat[:, 2:3], mcol[:])
                nc.vector.tensor_copy(colmat[:, 3:4], mcol[:])
                psf = psF.tile([4, 4], F32)
                nc.tensor.matmul(psf[:], mask_sb[:], colmat[:], start=True, stop=True)
                out_sb = pF.tile([4, 4], F32)
                nc.scalar.copy(out_sb[:], psf[:])
                nc.sync.dma_start(pout.ap()[:, :], out_sb[:])

    nc.compile()
    return nc


_NC_CACHE = {}


def get_nc():
    if "nc" not in _NC_CACHE:
        _NC_CACHE["nc"] = build_bass()
    return _NC_CACHE["nc"]


def make_core_inputs(embed_seq, embed_label, noise, sqrt_alphas_cumprod,
                     sqrt_one_minus_alphas_cumprod, Wq, Wk, Wv, Wo, timestep):
    bf = ml_dtypes.bfloat16
    f8 = ml_dtypes.float8_e4m3fn
    eyeX = np.zeros((P, P), dtype=np.float32)
    for i in range(32):
        eyeX[32 + i, i] = 1.0
        eyeX[96 + i, 64 + i] = 1.0
    eyeX4 = np.ascontiguousarray(
        np.broadcast_to(eyeX, (4, P, P)).transpose(1, 0, 2)).astype(bf)
    maskmat = np.zeros((P, 4), dtype=np.float32)
    maskmat[:, 0] = 1.0
    maskmat[:, 1] = 1.0
    maskmat[32:64, 2] = 1.0
    maskmat[96:128, 2] = 1.0
    maskmat[0:32, 3] = 1.0
    maskmat[64:96, 3] = 1.0

    sa_all = np.asarray(sqrt_alphas_cumprod)[np.asarray(timestep)].astype(np.float32)
    s1m_all = np.asarray(sqrt_one_minus_alphas_cumprod)[np.asarray(timestep)].astype(np.float32)

    es = np.asarray(embed_seq, dtype=np.float32)
    el = np.asarray(embed_label, dtype=np.float32)
    ns = np.asarray(noise, dtype=np.float32)
    wstack = (np.stack([np.asarray(w, dtype=np.float32)
                        for w in (Wq, Wk, Wv, Wo)]) * SW).astype(f8)  # [4, D, D]

    in_maps = []
    for c in range(NCORES):
        bsl = slice(c * BL, (c + 1) * BL)
        chsl = slice(c * CHL, (c + 1) * CHL)
        flT = np.ascontiguousarray(el[:, :, chsl].transpose(1, 2, 0))  # [L, CHL, B]
        fsT = np.ascontiguousarray(es[:, :, chsl].transpose(1, 2, 0))
        gT = np.ascontiguousarray(np.stack([flT, fsT], axis=2)).astype(f8)
        im = {
            "esT": np.ascontiguousarray(es[bsl].transpose(0, 2, 1)).astype(f8),
            "elT": np.ascontiguousarray(el[bsl].transpose(0, 2, 1)).astype(f8),
            "nsT": np.ascontiguousarray(ns[bsl].transpose(0, 2, 1)).astype(f8),
            "sa": sa_all[bsl].reshape(BL, 1).copy(),
            "s1m": s1m_all[bsl].reshape(BL, 1).copy(),
            "gT": gT,
            "eyeX4": eyeX4,
            "maskmat": maskmat,
        }
        if USE_ALLGATHER:
            im["wsh"] = np.ascontiguousarray(
                wstack[[0, 1, 3, 2], :, c * (D // NCORES):(c + 1) * (D // NCORES)])
        else:
            im["wfull"] = wstack
        in_maps.append(im)
    return in_maps


def combine_partials(partials, sqrt_alphas_cumprod):
    """partials: list of 8 [4,4] arrays; diag = [xsq, mse, match, ctr] sums."""
    xsq = sum(float(np.asarray(p)[0, 0]) for p in partials)
    mse = sum(float(np.asarray(p)[1, 1]) for p in partials)
    match = sum(float(np.asarray(p)[2, 2]) for p in partials)
    ctr = sum(float(np.asarray(p)[3, 3]) for p in partials)
    n_el = B * SEQ * D
    sa_T = float(np.asarray(sqrt_alphas_cumprod)[T - 1])
    loss = mse / n_el + (sa_T ** 2) * xsq / n_el + match / (D * B) + ctr / (D * B)
    return np.float32(loss)


def kernel(**inputs):
    nc = get_nc()
    in_maps = make_core_inputs(**inputs)
    res = run_bass_kernel_spmd(nc, in_maps, core_ids=list(range(NCORES)))
    partials = [res.results[c]["pout"] for c in range(NCORES)]
    return combine_partials(partials, inputs["sqrt_alphas_cumprod"])

